# revision 1
# baseline (speedup 1.0000x reference)
"""Trainium2 Bass kernel: C = triu(A @ B), A/B upper-triangular 4096x4096 fp32.

Strategy (row-parallel over 8 cores, SPMD single program):
  * 32 row-blocks of 128 rows. Core c owns blocks {c, 8+c, 16+c, 24+c}
    ("slot" j = block 8j + c).
  * One uniform schedule for all cores: for column tile q (8 tiles of 512)
    and slot j, accumulate k-tiles k in [8j, 4q+3].  Per-core variation
    lives entirely in the DATA: the host packs A^T tiles per core and
    zero-fills tiles with k < own-block, so padded matmuls contribute
    exact zeros.  Since A and B are both upper-triangular, the lower
    triangle of C comes out exactly 0 - no masking needed.
  * A^T pack (80 tiles of 128x128) is cached in SBUF; B streams once per
    column tile with below-diagonal tiles skipped.
"""

import numpy as np
from contextlib import ExitStack

import concourse.mybir as mybir
import concourse.tile as tile
from concourse import bacc, bass_utils

N = 4096
P = 128
NCORES = 8
NSLOT = 4          # row-block slots per core
NQ = 8             # 512-wide output column tiles
QW = 512
NKT = 32           # 128-wide k tiles
KSTART = [0, 8, 16, 24]            # first k-tile per slot (min over cores)
ANT = [32, 24, 16, 8]              # k-tiles stored per slot
AOFF = [0, 32, 56, 72]             # slot offsets into the A pack
ATOT = 80                          # total packed A tiles per core

# (slot, qtile) pairs the program computes/writes, in emission order
PAIRS = [(j, q) for q in range(NQ) for j in range(NSLOT) if 4 * q + 4 > 8 * j]
NT = len(PAIRS)                    # 20 output tiles of 128x512 per core

# matmul dtype mode: "fp32r" (fast, ~11-bit mantissa), "bf16x3" (hi/lo
# 3-pass split, near-fp32 accuracy), "fp32" (exact, 4x slower PE)
MODE = "fp32r"

# pool buffer counts (double/triple buffering)
BUFS_B = 3
BUFS_O = 4
BUFS_PS = 8

_nc_cache = {}


def build_nc(mode=MODE, rep=1, variant="full"):
    """rep>1 repeats the whole compute (for dispatch-overhead-cancelling
    timing): T_hw ~= (T(rep=R) - T(rep=1)) / (R-1).
    variant: "full" | "nomm" (DMAs only) | "nodma" (matmuls only)."""
    if (mode, rep, variant) in _nc_cache:
        return _nc_cache[(mode, rep, variant)]
    two = 2 if mode == "bf16x3" else 1
    dt_in = {
        "fp32r": mybir.dt.float32r,
        "bf16x3": mybir.dt.bfloat16,
        "fp32": mybir.dt.float32,
    }[mode]

    nc = bacc.Bacc("TRN2", target_bir_lowering=False, debug=False,
                   num_devices=NCORES)
    # partition-major packed layouts (see pack_inputs): per-partition data is
    # contiguous so every DMA is 128 descriptors of large contiguous runs.
    # Apack row = h*P + p(k-within-tile), col = t*P + m  (40KB/partition)
    a_dram = nc.dram_tensor("Apack", [two * P, ATOT * P], dt_in,
                            kind="ExternalInput").ap()
    # B row = (h*NQ + q)*P + p, col = k*QW + n          (8KB runs/partition)
    b_dram = nc.dram_tensor("B", [two * NQ * P, NKT * QW], dt_in,
                            kind="ExternalInput").ap()
    c_dram = nc.dram_tensor("Cout", [NT * P, QW], mybir.dt.float32,
                            kind="ExternalOutput").ap()

    with tile.TileContext(nc) as tc:
        with ExitStack() as ctx:
            apool = ctx.enter_context(tc.tile_pool(name="apool", bufs=1))
            bpool = ctx.enter_context(tc.tile_pool(name="bpool", bufs=BUFS_B))
            opool = ctx.enter_context(tc.tile_pool(name="opool", bufs=BUFS_O))
            pspool = ctx.enter_context(
                tc.tile_pool(name="pspool", bufs=BUFS_PS, space="PSUM"))

            do_bdma = variant in ("full", "nomm", "vbdma")
            do_mm = variant in ("full", "nodma", "vmm")
            do_copy = variant in ("full", "nomm", "nodma", "vcopy")
            do_store = variant in ("full", "nomm", "nodma", "vstore")

            # A load split so early matmuls are gated only by the tiles they
            # read: slot0 k0..7 (feeds q=1/q=0) lands in ~1.5us, the rest
            # overlaps with the B stream.
            a_sb = apool.tile([P, two, ATOT, P], dt_in)
            for t0, t1 in [(0, 8), (8, 32), (32, ATOT)]:
                for h in range(two):
                    nc.sync.dma_start(
                        a_sb[:, h, t0:t1, :],
                        a_dram[h * P:(h + 1) * P, t0 * P:t1 * P].rearrange(
                            "p (t m) -> p t m", m=P))

            # micro variants: per rep emit n tiny ops, skip the main loop
            micro = variant.startswith("vd") or variant in ("vgps8", "vdve8")
            if micro:
                n_ops = (8 if variant in ("vgps8", "vdve8")
                         else int(variant[2:]))
                mpool = ctx.enter_context(tc.tile_pool(name="mp", bufs=16))
                for r in range(rep):
                    for i in range(n_ops):
                        mt = mpool.tile([P, QW], mybir.dt.float32, tag="mt",
                                        name=f"mt_{r}_{i}")
                        if variant == "vdve8":
                            src = a_sb[:, 0, 4 * i:4 * i + 4, :]
                            if dt_in == mybir.dt.float32r:
                                src = src.bitcast(mybir.dt.float32)
                            nc.vector.tensor_copy(
                                mt[:].rearrange("p (a b) -> p a b", a=4),
                                src)
                        elif variant == "vgps8":
                            nc.gpsimd.dma_start(
                                mt[:],
                                b_dram[i * P:(i + 1) * P, 0:QW]
                                .bitcast(mybir.dt.float32))
                        else:
                            nc.sync.dma_start(
                                mt[:],
                                b_dram[i * P:(i + 1) * P, 0:QW]
                                .bitcast(mybir.dt.float32))
            bt_fixed = None
            ot_fixed = None

            def _asrc_f32(j):
                src = a_sb[:, 0, 4 * j:4 * j + 4, :]
                if dt_in == mybir.dt.float32r:
                    src = src.bitcast(mybir.dt.float32)
                return src

            if variant == "vstore":
                ot_fixed = opool.tile([P, QW], mybir.dt.float32,
                                      name="ot_fixed")
                nc.vector.tensor_copy(
                    ot_fixed[:].rearrange("p (a b) -> p a b", a=4),
                    _asrc_f32(0))

            def _bsrc(h, kg, q):
                return b_dram[
                    (h * NQ + q) * P:(h * NQ + q + 1) * P,
                    4 * kg * QW:(4 * kg + 4) * QW,
                ].rearrange("p (ko n) -> p ko n", ko=4)

            def _load_diag_chunk(bt, q):
                # per k-row load only the valid columns [128i, 512) -
                # below-diagonal 128-blocks of B are zero
                for h in range(two):
                    for i in range(4):
                        row = (h * NQ + q) * P
                        col = (4 * q + i) * QW + 128 * i
                        nc.sync.dma_start(
                            bt[:, h, i, 128 * i:],
                            b_dram[row:row + P, col:col + QW - 128 * i])

            # q=0's only chunk (0.6MB) is consumed last (Q_ORDER ends on 0):
            # prefetch it into a dedicated buffer at the start so the tail
            # never waits on DMA
            # (tried: prefetching q=0's chunk at the head — model-worse by
            # 1.8us, the DMA stream is saturated so early bytes displace
            # the critical sequence)
            bt_q0 = None

            # q order: q=1 first (ready after the small A-head load), then
            # heaviest-to-lightest so the schedule drains into the tiny q=0
            # tail (4 matmuls + 1 copy + 1 store). Model-swept optimum.
            Q_ORDER = globals().get("_Q_ORDER_OVERRIDE") or \
                [1, 7, 6, 5, 4, 3, 2, 0]
            for _r, q in ([] if micro else
                          [(r, q) for r in range(rep) for q in Q_ORDER]):
                act = [j for j in range(NSLOT) if 4 * q + 4 > 8 * j]
                psums = {
                    j: pspool.tile([P, QW], mybir.dt.float32, tag="ps",
                                   name=f"ps_{_r}_{q}_{j}")
                    for j in act
                } if do_mm else {}
                kend = 4 * q + 3
                for kg in range(q + 1):
                    if do_mm and not do_bdma:
                        if bt_fixed is None:
                            bt_fixed = bpool.tile([P, two, 4, QW], dt_in,
                                                  tag="bt", name="bt_fixed")
                            for h in range(two):
                                nc.sync.dma_start(bt_fixed[:, h],
                                                  _bsrc(h, 0, 0))
                        bt = bt_fixed
                    elif do_bdma or variant == "vmin":
                        if variant == "vmin" and kg > 0:
                            continue
                        if bt_q0 is not None and q == 0:
                            bt = bt_q0
                        else:
                            bt = bpool.tile([P, two, 4, QW], dt_in,
                                            tag="bt")
                            if kg == q:
                                _load_diag_chunk(bt, q)
                            else:
                                for h in range(two):
                                    nc.sync.dma_start(bt[:, h],
                                                      _bsrc(h, kg, q))
                    else:
                        continue
                    if not do_mm:
                        continue
                    for i in range(4):
                        k = 4 * kg + i
                        # on the diagonal chunk only columns >= 128i are
                        # valid in SBUF (and B is zero left of them anyway)
                        c0 = 128 * i if kg == q else 0
                        for j in act:
                            if k < KSTART[j]:
                                continue
                            idx = AOFF[j] + (k - KSTART[j])
                            first = k == KSTART[j]
                            last = k == kend
                            if two == 1:
                                nc.tensor.matmul(
                                    psums[j][:, c0:], a_sb[:, 0, idx, :],
                                    bt[:, 0, i, c0:],
                                    start=first, stop=last)
                            else:
                                # hi@hi, hi@lo, lo@hi
                                for n3, (ha, hb) in enumerate(
                                        [(0, 0), (0, 1), (1, 0)]):
                                    nc.tensor.matmul(
                                        psums[j][:, c0:],
                                        a_sb[:, ha, idx, :],
                                        bt[:, hb, i, c0:],
                                        start=first and n3 == 0,
                                        stop=last and n3 == 2)
                for j in act:
                    if not (do_copy or do_store):
                        continue
                    t = PAIRS.index((j, q))
                    if variant == "vstore":
                        nc.sync.dma_start(
                            c_dram[t * P:(t + 1) * P, :], ot_fixed[:])
                        continue
                    ot = opool.tile([P, QW], mybir.dt.float32, tag="ot")
                    if do_mm:
                        nc.vector.tensor_copy(ot[:], psums[j][:])
                    else:
                        nc.vector.tensor_copy(
                            ot[:].rearrange("p (a b) -> p a b", a=4),
                            _asrc_f32(j))
                    if do_store:
                        # scalar (ACT) HWDGE ring: keeps compute-gated output
                        # stores out of the B-stream's SP FIFO
                        nc.scalar.dma_start(
                            c_dram[t * P:(t + 1) * P, :], ot[:])
    nc.compile()
    _nc_cache[(mode, rep, variant)] = nc
    return nc


def _split_bf16(x):
    import ml_dtypes
    hi = x.astype(ml_dtypes.bfloat16)
    lo = (x - hi.astype(np.float32)).astype(ml_dtypes.bfloat16)
    return hi, lo


def pack_inputs(A, B, mode=MODE):
    """Build per-core in_maps (partition-major packed layouts)."""
    A = np.ascontiguousarray(np.asarray(A, dtype=np.float32))
    B = np.ascontiguousarray(np.asarray(B, dtype=np.float32))
    two = 2 if mode == "bf16x3" else 1

    # B[128k+p, 512q+n] -> Bp[q, p, k, n] -> [NQ*P, NKT*QW]
    def _pack_b(x):
        return np.ascontiguousarray(
            x.reshape(NKT, P, NQ, QW).transpose(2, 1, 0, 3)
        ).reshape(NQ * P, NKT * QW)

    if mode == "bf16x3":
        hi, lo = _split_bf16(B)
        b_all = np.concatenate([_pack_b(hi), _pack_b(lo)], axis=0)
    else:
        b_all = _pack_b(B)

    in_maps = []
    for c in range(NCORES):
        ap = np.zeros((ATOT, P, P), np.float32)
        for j in range(NSLOT):
            b = 8 * j + c
            rb = P * b
            for k in range(max(KSTART[j], b), NKT):
                ap[AOFF[j] + k - KSTART[j]] = \
                    A[rb:rb + P, P * k:P * k + P].T
        # [t, p, m] -> [p, t, m] -> [P, ATOT*P]
        def _pack_a(x):
            return np.ascontiguousarray(
                x.transpose(1, 0, 2)).reshape(P, ATOT * P)

        if mode == "bf16x3":
            hi, lo = _split_bf16(ap)
            apk = np.concatenate([_pack_a(hi), _pack_a(lo)], axis=0)
        else:
            apk = _pack_a(ap)
        in_maps.append({"Apack": apk, "B": b_all})
    return in_maps


def unpack_output(results):
    C = np.zeros((N, N), np.float32)
    for c, r in enumerate(results):
        co = np.asarray(r["Cout"]).reshape(NT, P, QW)
        for t, (j, q) in enumerate(PAIRS):
            b = 8 * j + c
            C[P * b:P * b + P, QW * q:QW * q + QW] = co[t]
    return C


def kernel(A, B):
    nc = build_nc(MODE)
    in_maps = pack_inputs(A, B, MODE)
    res = bass_utils.run_bass_kernel_spmd(
        nc, in_maps, core_ids=list(range(NCORES)), trace=False)
    return unpack_output(res.results)



# revision 11
# speedup vs baseline: 1.9840x; 1.9840x over previous
"""Trainium2 Bass kernel: C = triu(A @ B), A/B upper-triangular 4096x4096 fp32.

Strategy (row-parallel over 8 cores, SPMD single program):
  * 32 row-blocks of 128 rows. Core c owns blocks {c, 8+c, 16+c, 24+c}
    ("slot" j = block 8j + c).
  * One uniform schedule for all cores: for column tile q (8 tiles of 512)
    and slot j, accumulate k-tiles k in [8j, 4q+3].  Per-core variation
    lives entirely in the DATA: the host packs A^T tiles per core and
    zero-fills tiles with k < own-block, so padded matmuls contribute
    exact zeros.  Since A and B are both upper-triangular, the lower
    triangle of C comes out exactly 0 - no masking needed.
  * A^T pack (80 tiles of 128x128) is cached in SBUF; B streams once per
    column tile with below-diagonal tiles skipped.
"""

import numpy as np
from contextlib import ExitStack

import concourse.mybir as mybir
import concourse.tile as tile
from concourse import bacc, bass_utils

N = 4096
P = 128
NCORES = 8
NSLOT = 4          # row-block slots per core
NQ = 8             # 512-wide output column tiles
QW = 512
NKT = 32           # 128-wide k tiles
KSTART = [0, 8, 16, 24]            # first k-tile per slot (min over cores)
ANT = [32, 24, 16, 8]              # k-tiles stored per slot
AOFF = [0, 32, 56, 72]             # slot offsets into the A pack
ATOT = 80                          # total packed A tiles per core

# (slot, qtile) pairs the program computes/writes, in emission order
PAIRS = [(j, q) for q in range(NQ) for j in range(NSLOT) if 4 * q + 4 > 8 * j]
NT = len(PAIRS)                    # 20 output tiles of 128x512 per core

# matmul dtype mode: "fp32r" (fast, ~11-bit mantissa), "bf16" (single-pass
# bf16 in/out, halves DMA bytes, rel err ~3e-3), "bf16x3" (hi/lo 3-pass
# split, near-fp32 accuracy), "fp32" (exact, 4x slower PE)
MODE = "bf16"

# pool buffer counts (double/triple buffering)
BUFS_B = 8
BUFS_O = 4
BUFS_PS = 8

_nc_cache = {}


def build_nc(mode=MODE, rep=1, variant="full"):
    """rep>1 repeats the whole compute (for dispatch-overhead-cancelling
    timing): T_hw ~= (T(rep=R) - T(rep=1)) / (R-1).
    variant: "full" | "nomm" (DMAs only) | "nodma" (matmuls only)."""
    if (mode, rep, variant) in _nc_cache:
        return _nc_cache[(mode, rep, variant)]
    two = 2 if mode == "bf16x3" else 1
    dt_in = {
        "fp32r": mybir.dt.float32r,
        "bf16": mybir.dt.bfloat16,
        "bf16x3": mybir.dt.bfloat16,
        "fp32": mybir.dt.float32,
    }[mode]
    dt_out = mybir.dt.bfloat16 if mode == "bf16" else mybir.dt.float32

    nc = bacc.Bacc("TRN2", target_bir_lowering=False, debug=False,
                   num_devices=NCORES)
    # partition-major packed layouts (see pack_inputs): per-partition data is
    # contiguous so every DMA is 128 descriptors of large contiguous runs.
    # Apack row = h*P + p(k-within-tile), col = t*P + m  (40KB/partition)
    a_dram = nc.dram_tensor("Apack", [two * P, ATOT * P], dt_in,
                            kind="ExternalInput").ap()
    # B row = (h*NQ + q)*P + p, col = k*QW + n          (8KB runs/partition)
    b_dram = nc.dram_tensor("B", [two * NQ * P, NKT * QW], dt_in,
                            kind="ExternalInput").ap()
    c_dram = nc.dram_tensor("Cout", [NT * P, QW], dt_out,
                            kind="ExternalOutput").ap()

    with tile.TileContext(nc) as tc:
        with ExitStack() as ctx:
            apool = ctx.enter_context(tc.tile_pool(name="apool", bufs=1))
            bpool = ctx.enter_context(tc.tile_pool(name="bpool", bufs=BUFS_B))
            opool = ctx.enter_context(tc.tile_pool(name="opool", bufs=BUFS_O))
            pspool = ctx.enter_context(
                tc.tile_pool(name="pspool", bufs=BUFS_PS, space="PSUM"))

            do_bdma = variant in ("full", "nomm", "vbdma")
            do_mm = variant in ("full", "nodma", "vmm")
            do_copy = variant in ("full", "nomm", "nodma", "vcopy")
            do_store = variant in ("full", "nomm", "nodma", "vstore")

            # A load split so early matmuls are gated only by the tiles they
            # read: slot0 k0..7 (feeds q=1/q=0) lands in ~1.5us, the rest
            # overlaps with the B stream.
            a_sb = apool.tile([P, two, ATOT, P], dt_in)
            for t0, t1 in [(0, 8), (8, 32), (32, ATOT)]:
                for h in range(two):
                    nc.sync.dma_start(
                        a_sb[:, h, t0:t1, :],
                        a_dram[h * P:(h + 1) * P, t0 * P:t1 * P].rearrange(
                            "p (t m) -> p t m", m=P))

            # micro variants: per rep emit n tiny ops, skip the main loop
            micro = variant.startswith("vd") or variant in ("vgps8", "vdve8")
            if micro:
                n_ops = (8 if variant in ("vgps8", "vdve8")
                         else int(variant[2:]))
                mpool = ctx.enter_context(tc.tile_pool(name="mp", bufs=16))
                for r in range(rep):
                    for i in range(n_ops):
                        mt = mpool.tile([P, QW], mybir.dt.float32, tag="mt",
                                        name=f"mt_{r}_{i}")
                        if variant == "vdve8":
                            src = a_sb[:, 0, 4 * i:4 * i + 4, :]
                            if dt_in == mybir.dt.float32r:
                                src = src.bitcast(mybir.dt.float32)
                            nc.vector.tensor_copy(
                                mt[:].rearrange("p (a b) -> p a b", a=4),
                                src)
                        elif variant == "vgps8":
                            nc.gpsimd.dma_start(
                                mt[:],
                                b_dram[i * P:(i + 1) * P, 0:QW]
                                .bitcast(mybir.dt.float32))
                        else:
                            nc.sync.dma_start(
                                mt[:],
                                b_dram[i * P:(i + 1) * P, 0:QW]
                                .bitcast(mybir.dt.float32))
            bt_fixed = None
            ot_fixed = None

            def _asrc_f32(j):
                src = a_sb[:, 0, 4 * j:4 * j + 4, :]
                if dt_in == mybir.dt.float32r:
                    src = src.bitcast(mybir.dt.float32)
                return src

            if variant == "vstore":
                ot_fixed = opool.tile([P, QW], dt_out,
                                      name="ot_fixed")
                nc.vector.tensor_copy(
                    ot_fixed[:].rearrange("p (a b) -> p a b", a=4),
                    _asrc_f32(0))

            def _bsrc(h, kg, q):
                return b_dram[
                    (h * NQ + q) * P:(h * NQ + q + 1) * P,
                    4 * kg * QW:(4 * kg + 4) * QW,
                ].rearrange("p (ko n) -> p ko n", ko=4)

            def _load_diag_chunk(bt, q):
                # per k-row load only the valid columns [128i, 512) -
                # below-diagonal 128-blocks of B are zero
                for h in range(two):
                    for i in range(4):
                        row = (h * NQ + q) * P
                        col = (4 * q + i) * QW + 128 * i
                        nc.sync.dma_start(
                            bt[:, h, i, 128 * i:],
                            b_dram[row:row + P, col:col + QW - 128 * i])

            # q=0's only chunk (0.6MB) is consumed last (Q_ORDER ends on 0):
            # prefetch it into a dedicated buffer at the start so the tail
            # never waits on DMA
            # (tried: prefetching q=0's chunk at the head — model-worse by
            # 1.8us, the DMA stream is saturated so early bytes displace
            # the critical sequence)
            bt_q0 = None

            # q order: q=1 first (ready after the small A-head load), then
            # heaviest-to-lightest so the schedule drains into the tiny q=0
            # tail (4 matmuls + 1 copy + 1 store). Model-swept optimum.
            Q_ORDER = globals().get("_Q_ORDER_OVERRIDE") or \
                [1, 2, 6, 7, 5, 4, 3, 0]
            for _r, q in ([] if micro else
                          [(r, q) for r in range(rep) for q in Q_ORDER]):
                act = [j for j in range(NSLOT) if 4 * q + 4 > 8 * j]
                psums = {
                    j: pspool.tile([P, QW], mybir.dt.float32, tag="ps",
                                   name=f"ps_{_r}_{q}_{j}")
                    for j in act
                } if do_mm else {}
                kend = 4 * q + 3
                for kg in range(q + 1):
                    if do_mm and not do_bdma:
                        if bt_fixed is None:
                            bt_fixed = bpool.tile([P, two, 4, QW], dt_in,
                                                  tag="bt", name="bt_fixed")
                            for h in range(two):
                                nc.sync.dma_start(bt_fixed[:, h],
                                                  _bsrc(h, 0, 0))
                        bt = bt_fixed
                    elif do_bdma or variant == "vmin":
                        if variant == "vmin" and kg > 0:
                            continue
                        if bt_q0 is not None and q == 0:
                            bt = bt_q0
                        else:
                            bt = bpool.tile([P, two, 4, QW], dt_in,
                                            tag="bt")
                            if kg == q:
                                _load_diag_chunk(bt, q)
                            else:
                                for h in range(two):
                                    nc.sync.dma_start(bt[:, h],
                                                      _bsrc(h, kg, q))
                    else:
                        continue
                    if not do_mm:
                        continue
                    for i in range(4):
                        k = 4 * kg + i
                        # on the diagonal chunk only columns >= 128i are
                        # valid in SBUF (and B is zero left of them anyway)
                        c0 = 128 * i if kg == q else 0
                        for j in act:
                            if k < KSTART[j]:
                                continue
                            idx = AOFF[j] + (k - KSTART[j])
                            first = k == KSTART[j]
                            last = k == kend
                            if two == 1:
                                nc.tensor.matmul(
                                    psums[j][:, c0:], a_sb[:, 0, idx, :],
                                    bt[:, 0, i, c0:],
                                    start=first, stop=last)
                            else:
                                # hi@hi, hi@lo, lo@hi
                                for n3, (ha, hb) in enumerate(
                                        [(0, 0), (0, 1), (1, 0)]):
                                    nc.tensor.matmul(
                                        psums[j][:, c0:],
                                        a_sb[:, ha, idx, :],
                                        bt[:, hb, i, c0:],
                                        start=first and n3 == 0,
                                        stop=last and n3 == 2)
                for j in act:
                    if not (do_copy or do_store):
                        continue
                    t = PAIRS.index((j, q))
                    if variant == "vstore":
                        nc.sync.dma_start(
                            c_dram[t * P:(t + 1) * P, :], ot_fixed[:])
                        continue
                    ot = opool.tile([P, QW], dt_out, tag="ot")
                    if do_mm:
                        nc.vector.tensor_copy(ot[:], psums[j][:])
                    else:
                        nc.vector.tensor_copy(
                            ot[:].rearrange("p (a b) -> p a b", a=4),
                            _asrc_f32(j))
                    if do_store:
                        # scalar (ACT) HWDGE ring: keeps compute-gated output
                        # stores out of the B-stream's SP FIFO
                        nc.scalar.dma_start(
                            c_dram[t * P:(t + 1) * P, :], ot[:])
    nc.compile()
    _nc_cache[(mode, rep, variant)] = nc
    return nc


def _split_bf16(x):
    import ml_dtypes
    hi = x.astype(ml_dtypes.bfloat16)
    lo = (x - hi.astype(np.float32)).astype(ml_dtypes.bfloat16)
    return hi, lo


def pack_inputs(A, B, mode=MODE):
    """Build per-core in_maps (partition-major packed layouts)."""
    A = np.ascontiguousarray(np.asarray(A, dtype=np.float32))
    B = np.ascontiguousarray(np.asarray(B, dtype=np.float32))
    two = 2 if mode == "bf16x3" else 1

    # B[128k+p, 512q+n] -> Bp[q, p, k, n] -> [NQ*P, NKT*QW]
    def _pack_b(x):
        return np.ascontiguousarray(
            x.reshape(NKT, P, NQ, QW).transpose(2, 1, 0, 3)
        ).reshape(NQ * P, NKT * QW)

    if mode == "bf16x3":
        hi, lo = _split_bf16(B)
        b_all = np.concatenate([_pack_b(hi), _pack_b(lo)], axis=0)
    elif mode == "bf16":
        import ml_dtypes
        b_all = _pack_b(B).astype(ml_dtypes.bfloat16)
    else:
        b_all = _pack_b(B)

    in_maps = []
    for c in range(NCORES):
        ap = np.zeros((ATOT, P, P), np.float32)
        for j in range(NSLOT):
            b = 8 * j + c
            rb = P * b
            for k in range(max(KSTART[j], b), NKT):
                ap[AOFF[j] + k - KSTART[j]] = \
                    A[rb:rb + P, P * k:P * k + P].T
        # [t, p, m] -> [p, t, m] -> [P, ATOT*P]
        def _pack_a(x):
            return np.ascontiguousarray(
                x.transpose(1, 0, 2)).reshape(P, ATOT * P)

        if mode == "bf16x3":
            hi, lo = _split_bf16(ap)
            apk = np.concatenate([_pack_a(hi), _pack_a(lo)], axis=0)
        elif mode == "bf16":
            import ml_dtypes
            apk = _pack_a(ap).astype(ml_dtypes.bfloat16)
        else:
            apk = _pack_a(ap)
        in_maps.append({"Apack": apk, "B": b_all})
    return in_maps


def unpack_output(results):
    C = np.zeros((N, N), np.float32)
    for c, r in enumerate(results):
        co = np.asarray(r["Cout"]).astype(np.float32).reshape(NT, P, QW)
        for t, (j, q) in enumerate(PAIRS):
            b = 8 * j + c
            C[P * b:P * b + P, QW * q:QW * q + QW] = co[t]
    return C


def kernel(A, B):
    nc = build_nc(MODE)
    in_maps = pack_inputs(A, B, MODE)
    res = bass_utils.run_bass_kernel_spmd(
        nc, in_maps, core_ids=list(range(NCORES)), trace=False)
    return unpack_output(res.results)



# revision 13
# speedup vs baseline: 2.1000x; 1.0585x over previous
"""v3: 4 row-groups x 2 column-halves at 256-col granularity.

C = triu(A @ B), 4096^2, bf16.  Cores (r, s): r = c % 4, s = c // 4.
16 half-columns q' (256 cols, depth 2q'+2 k-tiles).  s=0 takes odd q',
s=1 even.  Env col e = 0..7 pairs ranks: s=0 -> q' = 15-2e (depth
32-4e = DEPTH[e]), s=1 -> q' = 14-2e (depth 30-4e, front-padded 2 zero
k-steps so its diagonal taper aligns with the schedule's).
Slots j = 0..7, block b = 4j + r, live iff 4j < DEPTH[e].
"""

import numpy as np
from contextlib import ExitStack

import concourse.mybir as mybir
import concourse.tile as tile
from concourse import bacc, bass_utils

N = 4096
P = 128
NCORES = 8
CW = 256
NENV = 8
DEPTH = [32 - 4 * e for e in range(NENV)]
NSLOT = 8
PAIRS = [(e, j) for e in range(NENV) for j in range(NSLOT)
         if 4 * j < DEPTH[e]]
NT = len(PAIRS)                          # 36 output tiles per core
# A-pack, w-major: position(w, j) = PRE[w] + j for j <= w//4
PRE = np.cumsum([0] + [w // 4 + 1 for w in range(32)]).tolist()
ATOT = PRE[32]                           # 144 tiles
ACH = [(PRE[4 * kc], PRE[4 * kc + 4]) for kc in range(8)]

BUFS_B = 10
BUFS_O = 6
BUFS_PS = 8

# env col emission order; first FWD_COLS run forward-k.  Ascending size
# start (small DMA deficit while PE ramps), deep cols last (PE-rich per
# byte, so the tail is compute-bound, not DMA-bound).
I_ORDER = [6, 5, 4, 3, 2, 7, 1, 0]
FWD_COLS = 0

MODE = "bf16"
_nc_cache = {}


def build_nc(mode=None, rep=1, variant="full"):
    if mode in (1, 2, 4, 8):
        rep, mode = mode, None
    key = (rep, variant, tuple(I_ORDER), FWD_COLS)
    if key in _nc_cache:
        return _nc_cache[key]
    dt_in = mybir.dt.bfloat16
    dt_out = mybir.dt.bfloat16

    nc = bacc.Bacc("TRN2", target_bir_lowering=False, debug=False,
                   num_devices=NCORES)
    a_dram = nc.dram_tensor("Apack", [P, ATOT * P], dt_in,
                            kind="ExternalInput").ap()
    # B-pack row = e*P + p, col = w*CW + n  (w local to env col)
    b_dram = nc.dram_tensor("B", [NENV * P, 32 * CW], dt_in,
                            kind="ExternalInput").ap()
    c_dram = nc.dram_tensor("Cout", [NT * P, CW], dt_out,
                            kind="ExternalOutput").ap()

    with tile.TileContext(nc) as tc:
        with ExitStack() as ctx:
            apool = ctx.enter_context(tc.tile_pool(name="apool", bufs=1))
            bpool = ctx.enter_context(tc.tile_pool(name="bpool", bufs=BUFS_B))
            opool = ctx.enter_context(tc.tile_pool(name="opool", bufs=BUFS_O))
            pspool = ctx.enter_context(
                tc.tile_pool(name="pspool", bufs=BUFS_PS, space="PSUM"))

            do_bdma = variant in ("full", "nomm")
            do_mm = variant in ("full", "nodma")

            a_sb = apool.tile([P, ATOT, P], dt_in)

            # per-col plan: groups of up to 2 chunks in processing order
            plan = []
            seen_a = set()
            for ii, e in enumerate(I_ORDER):
                ngk = DEPTH[e] // 4
                fwd = ii < FWD_COLS
                kgs = list(range(ngk)) if fwd \
                    else list(range(ngk - 1, -1, -1))
                groups = []
                for x in range(0, len(kgs), 2):
                    grp = kgs[x:x + 2]
                    anew = [kg for kg in grp if kg not in seen_a]
                    seen_a.update(anew)
                    groups.append((grp, sorted(anew)))
                plan.append((e, ngk, fwd, groups))

            def _issue_a_chunk(kc):
                t0, t1 = ACH[kc]
                nc.sync.dma_start(
                    a_sb[:, t0:t1, :],
                    a_dram[:, t0 * P:t1 * P].rearrange(
                        "p (t m) -> p t m", m=P))

            def _bw(e, w0, w1):
                # B window [w0, w1) of env col e as one DMA source
                return b_dram[
                    e * P:(e + 1) * P,
                    w0 * CW:w1 * CW,
                ].rearrange("p (ko n) -> p ko n", ko=w1 - w0)

            def _emit_pair(e, jpair, pst, do_mm, lane0=0):
                """One copy + one store for slots jpair sharing a psum
                bank.  jpair ascending; lanes t-ascending by layout.
                lane0: psum lane of jpair[0]."""
                nlane = len(jpair)
                t0 = PAIRS.index((e, jpair[0]))
                ot = opool.tile([P, nlane, CW], dt_out, tag=f"ot{nlane}")
                if do_mm:
                    nc.vector.tensor_copy(
                        ot[:], pst[:, lane0:lane0 + nlane])
                else:
                    nc.vector.tensor_copy(
                        ot[:].rearrange("p g (a b) -> p g a b", a=2),
                        a_sb[:, :2 * nlane, :].rearrange(
                            "p (g a) m -> p g a m", g=nlane))
                if nlane == 2:
                    nc.gpsimd.dma_start(
                        c_dram[t0 * P:(t0 + 2) * P, :].rearrange(
                            "(g p) n -> p g n", g=2), ot[:])
                else:
                    nc.gpsimd.dma_start(
                        c_dram[t0 * P:(t0 + 1) * P, :], ot[:, 0])

            bts = {}       # (ci, gi) -> loaded B tile
            bt_fixed = [None]

            def _load_group(ci, gi, first=False):
                if (ci, gi) in bts:
                    return bts[(ci, gi)]
                e, ngk, fwd, groups = plan[ci]
                grp, anew = groups[gi]
                # A chunk feeding the group's first-processed chunk goes
                # before the B load; the rest after (off the critical path)
                a_pre = [kc for kc in anew if kc == grp[0]]
                a_post = [kc for kc in anew if kc != grp[0]]
                for kc in a_pre:
                    _issue_a_chunk(kc)
                w0 = 4 * min(grp)
                w1 = 4 * max(grp) + 4
                if not do_bdma:
                    if bt_fixed[0] is None:
                        bt_fixed[0] = bpool.tile([P, 8, CW], dt_in,
                                                 tag="bt8")
                        nc.sync.dma_start(bt_fixed[0][:], _bw(e, 0, 8))
                    bts[(ci, gi)] = bt_fixed[0]
                    for kc in a_post:
                        _issue_a_chunk(kc)
                    return bt_fixed[0]
                bt = bpool.tile([P, w1 - w0, CW], dt_in,
                                tag=f"bt{w1 - w0}")
                isdiag = max(grp) == ngk - 1
                if first:
                    # first two k-tiles individually (first matmul waits on
                    # 64KB), rest as one load
                    for u in range(2):
                        nc.sync.dma_start(
                            bt[:, u, :],
                            b_dram[e * P:(e + 1) * P,
                                   (w0 + u) * CW:(w0 + u + 1) * CW])
                    last = w1 - 1 - w0
                    if isdiag:
                        nc.sync.dma_start(bt[:, 2:last, :],
                                          _bw(e, w0 + 2, w1 - 1))
                        nc.sync.dma_start(
                            bt[:, last, 128:],
                            b_dram[e * P:(e + 1) * P,
                                   (w1 - 1) * CW + 128:w1 * CW])
                    else:
                        nc.sync.dma_start(bt[:, 2:, :], _bw(e, w0 + 2, w1))
                elif isdiag:
                    # top k-tile's low 128 cols are below-diag zeros
                    nc.sync.dma_start(bt[:, :w1 - 1 - w0, :],
                                      _bw(e, w0, w1 - 1))
                    nc.sync.dma_start(
                        bt[:, w1 - 1 - w0, 128:],
                        b_dram[e * P:(e + 1) * P,
                               (w1 - 1) * CW + 128:w1 * CW])
                else:
                    nc.sync.dma_start(bt[:], _bw(e, w0, w1))
                bts[(ci, gi)] = bt
                for kc in a_post:
                    _issue_a_chunk(kc)
                return bt

            for _r in range(rep):
                for ci, (e, ngk, fwd, groups) in enumerate(plan):
                    nk = DEPTH[e]
                    act = [j for j in range(NSLOT) if 4 * j < nk]
                    # slot pairs = chunk groups: each pair shares one
                    # [P, 2, CW] psum tile (= one 2KB bank).  start=True
                    # zeroes the whole 2KB zero-region and stop clears its
                    # started flag, so exactly ONE start (first matmul into
                    # the bank, zeroing both lanes) and ONE stop (last
                    # matmul into the bank) per pair.
                    jp = {}
                    pstile = {}
                    pcnt = {}   # pair -> total matmuls
                    pdone = {}  # pair -> matmuls emitted
                    for grp, _ in groups:
                        pair = tuple(sorted(grp))
                        for j in pair:
                            jp[j] = (pair, j - pair[0])
                        pcnt[pair] = sum(
                            1 for kg in range(ngk) for u in range(4)
                            for j in pair if 4 * kg + u >= 4 * j)
                        pdone[pair] = 0
                        if do_mm:
                            pstile[pair] = pspool.tile(
                                [P, 2, CW], mybir.dt.float32, tag="ps",
                                name=f"ps_{_r}_{e}_{pair[0]}")
                    for gi, (grp, _) in enumerate(groups):
                        bt = _load_group(ci, gi)
                        if gi == min(1, len(groups) - 1) and _r == 0 \
                                and ci + 1 < len(plan):
                            # prefetch next col's top group + its A chunk
                            # mid-column, hiding the col-transition load
                            _load_group(ci + 1, 0)
                        w0 = 4 * min(grp)
                        for kg in grp:
                            if do_mm:
                                for u in range(4):
                                    w = 4 * kg + u
                                    c0 = 128 if (kg == ngk - 1 and u == 3) \
                                        else 0
                                    for j in act:
                                        if w < 4 * j:
                                            continue
                                        pair, lane = jp[j]
                                        nc.tensor.matmul(
                                            pstile[pair][:, lane, c0:],
                                            a_sb[:, PRE[w] + j, :],
                                            bt[:, w - w0, c0:],
                                            start=pdone[pair] == 0,
                                            stop=pdone[pair]
                                            == pcnt[pair] - 1)
                                        pdone[pair] += 1
                        if not fwd:
                            _emit_pair(e, tuple(sorted(grp)),
                                       pstile.get(tuple(sorted(grp))),
                                       do_mm)
                    if fwd:
                        for grp, _ in groups:
                            pair = tuple(sorted(grp))
                            _emit_pair(e, pair, pstile.get(pair), do_mm)
                    bts.pop((ci, 0), None)
    nc.compile()
    _nc_cache[key] = nc
    return nc


def _inst(c):
    """core -> per-env (q', kshift)"""
    r, s = c % 4, c // 4
    out = []
    for e in range(NENV):
        qp = (15 - 2 * e) if s == 0 else (14 - 2 * e)
        out.append((qp, 0 if s == 0 else 2))
    return r, out


def pack_inputs(A, B, mode=None):
    import ml_dtypes
    A = np.asarray(A, dtype=np.float32)
    B = np.asarray(B, dtype=np.float32)
    bf = ml_dtypes.bfloat16

    in_maps = []
    for c in range(NCORES):
        r, inst = _inst(c)
        ks = inst[0][1]  # kshift (same for all env cols of this core)
        # A pack: position PRE[w]+j holds A[block 4j+r, k=w-ks].T
        ap = np.zeros((ATOT, P, P), np.float32)
        for w in range(32):
            k = w - ks
            if not 0 <= k < 32:
                continue
            for j in range(w // 4 + 1):
                b = 4 * j + r
                if k >= b:
                    ap[PRE[w] + j] = \
                        A[P * b:P * b + P, P * k:P * k + P].T
        apk = np.ascontiguousarray(
            ap.transpose(1, 0, 2)).reshape(P, ATOT * P).astype(bf)

        bp = np.zeros((NENV, P, 32, CW), np.float32)
        for e, (qp, _) in enumerate(inst):
            for w in range(DEPTH[e]):
                k = w - ks
                if not 0 <= k < 32:
                    continue
                if P * k >= CW * (qp + 1):
                    continue  # below diagonal: zero
                bp[e, :, w, :] = \
                    B[P * k:P * k + P, CW * qp:CW * (qp + 1)]
        bpk = np.ascontiguousarray(
            bp.reshape(NENV * P, 32 * CW)).astype(bf)
        in_maps.append({"Apack": apk, "B": bpk})
    return in_maps


def unpack_output(results):
    C = np.zeros((N, N), np.float32)
    for c, r_ in enumerate(results):
        r, inst = _inst(c)
        co = np.asarray(r_["Cout"]).astype(np.float32).reshape(NT, P, CW)
        for t, (e, j) in enumerate(PAIRS):
            b = 4 * j + r
            qp = inst[e][0]
            if P * b >= CW * (qp + 1):
                continue  # entirely below diagonal
            C[P * b:P * b + P, CW * qp:CW * qp + CW] = co[t]
    return C


def kernel(A, B):
    nc = build_nc()
    in_maps = pack_inputs(A, B)
    res = bass_utils.run_bass_kernel_spmd(
        nc, in_maps, core_ids=list(range(NCORES)), trace=False)
    return unpack_output(res.results)
